# revision 2
# baseline (speedup 1.0000x reference)
"""Trainium2 Bass kernel for GQA attention (B=1, S=2048, D=2048, H=32, KV=8, HD=64).

Tensor-parallel over heads across 8 NeuronCores: core i holds q-heads
[4i, 4i+4) and kv-head i; each core computes its partial o_proj output and the
host sums the 8 partials (Megatron all-reduce done host-side).

v2: q-block-outer pipeline — for each 512-row s-block: project q/k/v, RoPE,
attention units for all 4 heads, normalization, and o_proj of that q-block.
This overlaps projection / softmax / o_proj across blocks so the tensor engine
never drains between phases.

Self-contained: only imports concourse (on sys.path in the container).
"""

import os
import sys

import ml_dtypes
import numpy as np

if "/opt/trn_rl_repo" not in sys.path and not any(
    p.endswith("trn_rl_repo") for p in sys.path
):
    sys.path.insert(0, "/opt/trn_rl_repo")

import concourse.bass as bass
import concourse.mybir as mybir
import concourse.tile as tile
from concourse import bacc
from concourse.bass_utils import run_bass_kernel_spmd
from concourse.masks import make_identity

F32 = mybir.dt.float32
BF16 = mybir.dt.bfloat16

AF = mybir.ActivationFunctionType
ALU = mybir.AluOpType

S = 2048
D = 2048
H = 32
KV = 8
HD = 64
NCORES = 8
HQ = H // NCORES  # 4 q heads per core
NSB = 4  # s blocks of 512
SBW = 512
DCH = D // 128  # 16 contraction chunks


def _build_nc():
    nc = bacc.Bacc("TRN2", target_bir_lowering=False, debug=False, num_devices=NCORES)

    xt_d = nc.declare_dram_parameter("xt", [D, S], BF16, isOutput=False)
    wqkv_d = nc.declare_dram_parameter("wqkv", [D, 384], BF16, isOutput=False)
    wo_d = nc.declare_dram_parameter("wo", [2, 128, D], BF16, isOutput=False)
    cos_d = nc.declare_dram_parameter("cos", [128, S], BF16, isOutput=False)
    sin_d = nc.declare_dram_parameter("sin", [128, S], BF16, isOutput=False)
    sel_d = nc.declare_dram_parameter("sel", [4, 4 * 128], BF16, isOutput=False)
    y_d = nc.declare_dram_parameter("y", [S, D], BF16, isOutput=True)

    with tile.TileContext(nc) as tc:
        with tc.tile_pool(name="glob", bufs=1) as glob:
            ktdup = glob.tile([128, S], BF16, tag="ktdup")
            v_s = glob.tile([128, DCH, 128], BF16, tag="v_s")
            ao = glob.tile([128, 2, S], BF16, tag="ao")
            sel_s = glob.tile([4, 4 * 128], BF16, tag="sel_s")
            ident = glob.tile([128, 128], F32, tag="ident")
            wo_s = glob.tile([128, 2, D], BF16, tag="wo_s")
            wq_s = glob.tile([128, DCH, 384], BF16, tag="wq_s")
            cos_s = glob.tile([128, S], BF16, tag="cos_s")
            sin_s = glob.tile([128, S], BF16, tag="sin_s")

            nc.vector.memset(v_s[:], 0.0)
            nc.vector.memset(v_s[:, :, 64], 1.0)

            with (
                tc.tile_pool(name="xp", bufs=2) as xp,
                tc.tile_pool(name="kvp", bufs=2) as kvp,
                tc.tile_pool(name="abp", bufs=2) as abp,
                tc.tile_pool(name="qsp", bufs=5) as qsp,
                tc.tile_pool(name="tmpp", bufs=4) as tmpp,
                tc.tile_pool(name="ptp", bufs=6) as ptp,
                tc.tile_pool(name="stgp", bufs=6) as stgp,
                tc.tile_pool(name="smp", bufs=2) as smp,
                tc.tile_pool(name="yp", bufs=2) as yp,
                tc.tile_pool(name="ps1", bufs=2, space="PSUM") as ps1,
                tc.tile_pool(name="pssc", bufs=2, space="PSUM") as pssc,
                tc.tile_pool(name="pso_p", bufs=2, space="PSUM") as pso_p,
            ):
                xt_r = xt_d.rearrange("(ko p) s -> p ko s", p=128)
                wqkv_r = wqkv_d.rearrange("(ko p) n -> p ko n", p=128)

                stg_of = {}  # (h, sb) -> stg65 tile

                def emit_unit(h, sb, qs, sums_sb):
                    """Attention unit for head h, q-block sb (512 q positions)."""
                    q0 = sb * SBW
                    nkc = 4 * (sb + 1)
                    pso = pso_p.tile([128, SBW], F32, tag="pso", name=f"pso_{h}_{sb}")
                    for pair in range(nkc // 2):
                        cA, cB = 2 * pair, 2 * pair + 1
                        psc = pssc.tile([128, 1024], F32, tag="psc", name=f"psc{h}{sb}{pair}")
                        ptt = ptp.tile([128, 1024], BF16, tag="ptt", name=f"ptt{h}{sb}{pair}")
                        for c, half, r0 in ((cA, 0, 0), (cB, 1, 64)):
                            kc0 = c * 128
                            d = max(0, kc0 - q0)
                            nc.tensor.matmul(
                                psc[:, half * 512 + d : half * 512 + 512],
                                lhsT=ktdup[r0 : r0 + 64, kc0 : kc0 + 128],
                                rhs=qs[r0 : r0 + 64, d:SBW],
                                start=True,
                                stop=True,
                                tile_position=(r0, 0),
                            )
                        dA = max(0, cA * 128 - q0)
                        # single exp over [dA:1024]; garbage in the invalid
                        # regions is zeroed by affine_select below.
                        nc.scalar.activation(ptt[:, dA:1024], psc[:, dA:1024], AF.Exp)
                        for c, half in ((cA, 0), (cB, 1)):
                            kc0 = c * 128
                            if kc0 + 127 > q0:
                                ww = min(512, (kc0 - q0) + 128)
                                sl = slice(half * 512, half * 512 + ww)
                                nc.gpsimd.affine_select(
                                    out=ptt[:, sl],
                                    in_=ptt[:, sl],
                                    compare_op=ALU.is_ge,
                                    fill=0.0,
                                    base=q0 - kc0,
                                    channel_multiplier=-1,
                                    pattern=[[1, ww]],
                                )
                        for c, half in ((cA, 0), (cB, 1)):
                            nc.tensor.matmul(
                                pso[:],
                                lhsT=v_s[:, c, :],
                                rhs=ptt[:, half * 512 : half * 512 + 512],
                                start=(c == 0),
                                stop=(c == nkc - 1),
                            )
                    # evict raw attn out + sums in one copy; row 64 = softmax sums
                    stg = stgp.tile([65, SBW], F32, tag="stg", name=f"stg_{h}_{sb}")
                    nc.vector.tensor_copy(stg[:], pso[0:65, :])
                    stg_of[(h, sb)] = stg
                    nc.gpsimd.dma_start(sums_sb[h : h + 1, :], stg[64:65, :])

                def normalize_sb(sb, sums_sb):
                    """Reciprocal of softmax sums + scale raw outputs into ao."""
                    rcp_f32 = smp.tile([4, SBW], F32, tag="rcp_f32", name=f"rf{sb}")
                    rcp_scr = smp.tile([4, SBW], F32, tag="rcp_scr", name=f"rs{sb}")
                    rcp_bf = smp.tile([4, SBW], BF16, tag="rcp_bf", name=f"rb{sb}")
                    nc.vector.reciprocal_approx_accurate(
                        rcp_f32[:], sums_sb[:], rcp_scr[:]
                    )
                    nc.vector.tensor_copy(rcp_bf[:], rcp_f32[:])
                    sbc = slice(sb * SBW, (sb + 1) * SBW)
                    for h in range(HQ):
                        ch = h // 2
                        rr = 64 * (h % 2)
                        pbc = pso_p.tile([128, SBW], F32, tag="pso", name=f"pbc{h}{sb}")
                        nc.tensor.matmul(
                            pbc[:],
                            lhsT=sel_s[:, h * 128 : (h + 1) * 128],
                            rhs=rcp_bf[:],
                            start=True,
                            stop=True,
                        )
                        stg = stg_of.pop((h, sb))
                        nc.vector.tensor_tensor(
                            ao[rr : rr + 64, ch, sbc],
                            stg[0:64, :],
                            pbc[rr : rr + 64, :],
                            ALU.mult,
                        )

                def o_proj_sb(sb):
                    """o_proj for q-block sb: y[sb*512:(sb+1)*512, :]."""
                    for stl in range(4):
                        st = 4 * sb + stl
                        ysb = yp.tile([128, D], BF16, tag="ysb", name=f"ysb{st}")
                        for ob in range(4):
                            psy = ps1.tile(
                                [128, SBW], F32, tag="proj", name=f"psy{st}{ob}"
                            )
                            for ch in range(2):
                                nc.tensor.matmul(
                                    psy[:],
                                    lhsT=ao[:, ch, st * 128 : (st + 1) * 128],
                                    rhs=wo_s[:, ch, ob * 512 : (ob + 1) * 512],
                                    start=(ch == 0),
                                    stop=(ch == 1),
                                )
                            osl = slice(ob * 512, (ob + 1) * 512)
                            if ob % 2 == 0:
                                nc.scalar.activation(ysb[:, osl], psy[:], AF.Copy)
                            else:
                                nc.vector.tensor_copy(ysb[:, osl], psy[:])
                        eng = nc.gpsimd if st % 2 == 0 else nc.sync
                        eng.dma_start(y_d[st * 128 : (st + 1) * 128, :], ysb[:])

                prev = None  # (sb, sums_sb) pending normalize+o_proj
                for sb in range(NSB):
                    sbc = slice(sb * SBW, (sb + 1) * SBW)
                    # ---- projections for s-block sb ----
                    xblk = xp.tile([128, DCH, SBW], BF16, tag="xblk", name=f"xb{sb}")
                    for kq in range(4):
                        if sb == 0:
                            for kc in range(4 * kq, 4 * kq + 4):
                                nc.sync.dma_start(wq_s[:, kc, :], wqkv_r[:, kc, :])
                        nc.sync.dma_start(
                            xblk[:, 4 * kq : 4 * kq + 4, :],
                            xt_r[:, 4 * kq : 4 * kq + 4, sbc],
                        )
                    if sb == 0:
                        nc.sync.dma_start(cos_s[:], cos_d[:])
                        nc.sync.dma_start(sin_s[:], sin_d[:])
                        nc.sync.dma_start(sel_s[:], sel_d[:])
                        for ch in range(2):
                            nc.sync.dma_start(wo_s[:, ch, :], wo_d[ch])
                        make_identity(nc, ident[:])
                    psKV = ps1.tile([128, SBW], F32, tag="proj", name=f"pKV{sb}")
                    psA = ps1.tile([128, SBW], F32, tag="proj", name=f"pA{sb}")
                    psB = ps1.tile([128, SBW], F32, tag="proj", name=f"pB{sb}")
                    for ps_t, col0 in ((psKV, 256), (psA, 0), (psB, 128)):
                        for kc in range(DCH):
                            nc.tensor.matmul(
                                ps_t[:],
                                lhsT=wq_s[:, kc, col0 : col0 + 128],
                                rhs=xblk[:, kc, :],
                                start=(kc == 0),
                                stop=(kc == DCH - 1),
                            )
                    # evict k|v rows early (frees the KV bank)
                    kvraw = kvp.tile([128, SBW], F32, tag="kvraw", name=f"kv{sb}")
                    nc.scalar.activation(kvraw[:], psKV[:], AF.Copy)

                    # RoPE on the 4 q heads (A = first-half dims, B = second)
                    outA = abp.tile([128, SBW], BF16, tag="outA", name=f"oA{sb}")
                    outB = abp.tile([128, SBW], BF16, tag="outB", name=f"oB{sb}")
                    tmp = tmpp.tile([128, SBW], F32, tag="tmp", name=f"t1{sb}")
                    nc.vector.tensor_tensor(outA[:], psA[:], cos_s[:, sbc], ALU.mult)
                    nc.vector.tensor_tensor(tmp[:], psB[:], sin_s[:, sbc], ALU.mult)
                    nc.vector.tensor_tensor(outA[:], outA[:], tmp[:], ALU.subtract)
                    tmp2 = tmpp.tile([128, SBW], F32, tag="tmp", name=f"t2{sb}")
                    nc.vector.tensor_tensor(outB[:], psB[:], cos_s[:, sbc], ALU.mult)
                    nc.vector.tensor_tensor(tmp2[:], psA[:], sin_s[:, sbc], ALU.mult)
                    nc.vector.tensor_tensor(outB[:], outB[:], tmp2[:], ALU.add)

                    # k RoPE on this s-block: kswap = [k_hi; k_lo]
                    kswap = kvp.tile([64, SBW], F32, tag="kswap", name=f"ks{sb}")
                    nc.sync.dma_start(kswap[0:32, :], kvraw[32:64, :])
                    nc.sync.dma_start(kswap[32:64, :], kvraw[0:32, :])
                    nc.vector.tensor_tensor(
                        ktdup[0:64, sbc], kvraw[0:64, :], cos_s[0:64, sbc], ALU.mult
                    )
                    tmpk = tmpp.tile([64, SBW], F32, tag="tmpk", name=f"tk{sb}")
                    nc.vector.tensor_tensor(tmpk[:], kswap[:], sin_s[0:64, sbc], ALU.mult)
                    nc.vector.tensor_tensor(
                        ktdup[0:32, sbc], ktdup[0:32, sbc], tmpk[0:32, :], ALU.subtract
                    )
                    nc.vector.tensor_tensor(
                        ktdup[32:64, sbc], ktdup[32:64, sbc], tmpk[32:64, :], ALU.add
                    )
                    nc.sync.dma_start(ktdup[64:128, sbc], ktdup[0:64, sbc])

                    # v: [64, 512] -> 4 key-chunk tiles [128, 64] via PE transpose
                    for cl in range(4):
                        c = 4 * sb + cl
                        ptr = pso_p.tile([128, SBW], F32, tag="pso", name=f"ptr{c}")
                        nc.tensor.transpose(
                            ptr[:, 0:64],
                            kvraw[64:128, cl * 128 : (cl + 1) * 128],
                            ident[64:128, 64:128],
                        )
                        nc.vector.tensor_copy(v_s[:, c, 0:64], ptr[:, 0:64])

                    # q streams for the 4 heads (duplicated rows for PE packing)
                    qs_h = []
                    for h in range(HQ):
                        hc = slice(32 * h, 32 * h + 32)
                        qs = qsp.tile([128, SBW], BF16, tag="qs", name=f"qs{h}_{sb}")
                        nc.sync.dma_start(qs[0:32, :], outA[hc, :])
                        nc.sync.dma_start(qs[32:64, :], outB[hc, :])
                        nc.sync.dma_start(qs[64:96, :], outA[hc, :])
                        nc.sync.dma_start(qs[96:128, :], outB[hc, :])
                        qs_h.append(qs)

                    # ---- finish previous block: normalize + o_proj ----
                    if prev is not None:
                        psb, psums = prev
                        normalize_sb(psb, psums)
                        o_proj_sb(psb)

                    # ---- attention units for this block ----
                    sums_sb = smp.tile([4, SBW], F32, tag="sums", name=f"sums{sb}")
                    for h in range(HQ):
                        emit_unit(h, sb, qs_h[h], sums_sb)
                    prev = (sb, sums_sb)

                psb, psums = prev
                normalize_sb(psb, psums)
                o_proj_sb(psb)

    nc.compile()
    return nc


def _prep_inputs(x, Wq, Wk, Wv, Wo, inv_freq):
    """Host-side sharding + layout prep. Returns in_maps for the 8 cores."""
    x = np.ascontiguousarray(np.asarray(x, dtype=np.float32).reshape(S, D))
    xt = np.ascontiguousarray(x.T)  # [D, S]

    pos = np.arange(S, dtype=np.float64)
    inv = np.asarray(inv_freq, dtype=np.float64)  # [32]
    freqs = pos[None, :] * inv[:, None]  # [32, S]
    cos32 = np.cos(freqs).astype(np.float32)
    sin32 = np.sin(freqs).astype(np.float32)
    cos_tab = np.tile(cos32, (4, 1))  # [128, S]
    sin_tab = np.tile(sin32, (4, 1))
    # sel[h, h*128 + 64*(h%2) : +64] = 1 broadcasts rcp row h to the ao rows
    # of head h (chunk h//2, row offset 64*(h%2)).
    sel = np.zeros((4, 4 * 128), dtype=np.float32)
    for h in range(HQ):
        rr = 64 * (h % 2)
        sel[h, h * 128 + rr : h * 128 + rr + 64] = 1.0

    in_maps = []
    for i in range(NCORES):
        wq_l = Wq[256 * i : 256 * (i + 1)].astype(np.float32) * 0.125  # [256, D]
        wk_l = Wk[64 * i : 64 * (i + 1)].astype(np.float32)  # [64, D]
        wv_l = Wv[64 * i : 64 * (i + 1)].astype(np.float32)  # [64, D]
        # A-tile: first-half dims of the 4 heads; B-tile: second halves
        wA = np.concatenate(
            [wq_l[64 * h : 64 * h + 32] for h in range(HQ)], axis=0
        )  # [128, D]
        wB = np.concatenate(
            [wq_l[64 * h + 32 : 64 * h + 64] for h in range(HQ)], axis=0
        )
        wkv = np.concatenate([wk_l, wv_l], axis=0)  # [128, D]
        wqkv = np.ascontiguousarray(
            np.concatenate([wA, wB, wkv], axis=0).T
        )  # [D, 384]
        wo_l = Wo[:, 256 * i : 256 * (i + 1)].astype(np.float32)  # [D, 256]
        wo_t = np.ascontiguousarray(wo_l.T.reshape(2, 128, D))  # [2, 128, D]
        in_maps.append(
            {
                "xt": xt.astype(ml_dtypes.bfloat16),
                "wqkv": wqkv.astype(ml_dtypes.bfloat16),
                "wo": wo_t.astype(ml_dtypes.bfloat16),
                "cos": cos_tab.astype(ml_dtypes.bfloat16),
                "sin": sin_tab.astype(ml_dtypes.bfloat16),
                "sel": sel.astype(ml_dtypes.bfloat16),
            }
        )
    return in_maps


_NC_CACHE = None


def kernel(x, Wq, Wk, Wv, Wo, inv_freq):
    global _NC_CACHE
    if _NC_CACHE is None:
        _NC_CACHE = _build_nc()
    nc = _NC_CACHE
    in_maps = _prep_inputs(x, Wq, Wk, Wv, Wo, inv_freq)
    trace = bool(int(os.environ.get("BASS_KERNEL_TRACE", "0")))
    res = None
    last_exc = None
    for attempt in range(3):
        try:
            res = run_bass_kernel_spmd(nc, in_maps, list(range(NCORES)), trace=trace)
            break
        except Exception as e:  # transient device faults (rare) — retry
            last_exc = e
            msg = str(e)
            if "UNRECOVERABLE" in msg or "UNAVAILABLE" in msg or "Timeout" in msg:
                continue
            raise
    if res is None:
        raise last_exc
    if trace:
        kernel.last_results = res
    y = np.zeros((S, D), dtype=np.float32)
    for i in range(NCORES):
        y += res.results[i]["y"].astype(np.float32)
    return y.reshape(1, S, D)
